# revision 8
# baseline (speedup 1.0000x reference)
"""DDSP generator Bass kernel for Trainium2, 8-core data parallel.

Sharding: batch 16 -> 8 cores x 2 examples each. Weights replicated.
Per core:
  stage1: main conv stack (fp32 PE) -> h; osc head -> l (amp^2), f (Hz/SR)
  osc bank, per 384-sample resize segment (plus two 192 edge segments):
      ACT lerp (Identity, per-partition scale/bias) ->
      custom DVE op (clip + cumsum + wrap to [-0.5, 0.5] cycles, one pass) ->
      ACT Sin -> fp16 -> m=2 PE reduce matmul with lhsT = [l_lo | dl],
      4 segments packed per PSUM bank via tile_position -> DVE copy -> DMA.
  noise branch: 4x (2x-upsample conv k7) via even/odd stride trick
      (host-combined 4-tap weights), fp16 matmuls (L1 fp32); head conv
      (duplicated 34-col weights) + Square -> n_l on partitions 0..33.
  noise FFT: rfft/irfft as DFT matmuls, filter on DVE, overlap-add.
Host: recombine the two reduce rows with the lerp-weight pattern, pad,
      add noise, normalize, crop (O(output) numpy work only).
"""

import numpy as np
from contextlib import ExitStack

import concourse.bass as bass
import concourse.tile as tile
from concourse import bacc, mybir
from concourse import bass_utils
from concourse import dve_ops
from concourse.dve_spec import Spec, Src0, Src1, C0, C1, C2, scan, minn, maxx, AluOp, lower
from concourse.dve_uop import DveOpSpec

F32 = mybir.dt.float32
F16 = mybir.dt.float16
AF = mybir.ActivationFunctionType
ALU = mybir.AluOpType

SR = 11025.0
UP_LEN = 24576
TOTAL = 16384
WIN = 32
FRAMES = 1024
CROP = 4096
B = 16
NCORES = 8
BPC = 2
T0 = 64
SEG = 384
NSEG = 63
EDGE = 192
NUNITS = NSEG + 2
LO_U = 20.0 / SR
HI_U = 0.5
MAGIC = 12582912.0

_CENTERS = np.geomspace(20.0, SR / 2.0 - 20.0, 128).astype(np.float32)
_ERBS = (_CENTERS * np.float32(0.108) + np.float32(24.7)).astype(np.float32)


def _osc_ref(in0, in1, s0, s1, imm2):
    v = np.minimum(np.maximum(in0, np.float32(s0)), np.float32(s1)).astype(np.float32)
    u = np.cumsum(v.astype(np.float64), axis=-1).astype(np.float32)
    y = (u + in1).astype(np.float32)
    r = ((y + np.float32(imm2)) - np.float32(imm2)).astype(np.float32)
    return (y - r).astype(np.float32)


def _register_osc_op():
    if hasattr(dve_ops, "CUSTOM_DVE_OPS_BY_NAME") and \
            "OSC_PHASE_ANT" in dve_ops.CUSTOM_DVE_OPS_BY_NAME:
        return dve_ops.CUSTOM_DVE_OPS_BY_NAME["OSC_PHASE_ANT"]
    body_v = minn(maxx(Src0, C0), C1)
    body_u = scan(AluOp.ADD, body_v)
    body_y = body_u + Src1
    body = body_y - ((body_y + C2) - C2)
    spec = Spec(body=body, reference=_osc_ref)
    sha = {}
    for ver in ("v3",):
        s = DveOpSpec(name="OSC_PHASE_ANT", opcode=1, uops=lower(spec, ver=ver),
                      rd1_en=True)
        sha[ver] = s.sha(ver)
    op = dve_ops.DveOp("OSC_PHASE_ANT", spec, subdim=False, uops_sha=sha)
    dve_ops.OPS.append(op)
    dve_ops.CUSTOM_DVE_SPECS[op.name] = op.spec
    dve_ops._SUB_OPCODE_FOR_NAME[op.name] = max(dve_ops._SUB_OPCODE_FOR_NAME.values()) + 1
    if not hasattr(dve_ops, "CUSTOM_DVE_OPS_BY_NAME"):
        dve_ops.CUSTOM_DVE_OPS_BY_NAME = {}
    dve_ops.CUSTOM_DVE_OPS_BY_NAME[op.name] = op
    return op


_BUILD_CACHE = {}


def _build_program():
    if "nc" in _BUILD_CACHE:
        return _BUILD_CACHE["nc"]
    osc_op = _register_osc_op()

    nc = bacc.Bacc("TRN2", target_bir_lowering=False, debug=False, num_devices=1)

    dI = lambda n, s, dt=F32: nc.dram_tensor(n, s, dt, kind="ExternalInput").ap()
    dO = lambda n, s, dt=F32: nc.dram_tensor(n, s, dt, kind="ExternalOutput").ap()

    x3 = dI("x3", [BPC, 256, T0])
    noi = dI("noi", [BPC, FRAMES, WIN])
    wm0 = dI("wm0", [256, 512])
    wmL = [dI(f"wm{i}", [512, 3, 512]) for i in (1, 2, 3)]
    wfq = dI("wfq", [512, 256])
    wn0 = dI("wn0", [2, 512, 4, 512], F16)                # [eo,cin,tap,cout]
    wnl = [dI(f"wnl{l}", [2, 512, 4, 512], F16) for l in (1, 2, 3)]
    wnh = dI("wnh", [512, 34], F16)                       # head, duplicated cols
    bnl = dI("bnl", [128, 16])
    wt = dI("wt", [128, SEG])
    fcat = dI("fcat", [WIN, 34])
    gmat = dI("gmat", [34, WIN])
    cesc = dI("cesc", [128, 1])
    cebi = dI("cebi", [128, 1])

    h_out = dO("h_out", [2 * BPC, UP_LEN])
    n_out = dO("n_out", [16 * BPC, FRAMES])

    with tile.TileContext(nc) as tc, ExitStack() as ctx:
        cpool = ctx.enter_context(tc.tile_pool(name="consts", bufs=1))
        apool = ctx.enter_context(tc.tile_pool(name="acts", bufs=1))
        fpool = ctx.enter_context(tc.tile_pool(name="fft", bufs=1))
        opool = ctx.enter_context(tc.tile_pool(name="osc", bufs=3))
        hpool = ctx.enter_context(tc.tile_pool(name="hm", bufs=2))
        w1pool = ctx.enter_context(tc.tile_pool(name="w1", bufs=2))
        w2pool = ctx.enter_context(tc.tile_pool(name="w2", bufs=2))
        ps_mm = ctx.enter_context(tc.tile_pool(name="psmm", bufs=3, space="PSUM"))
        ps_osc = ctx.enter_context(tc.tile_pool(name="psosc", bufs=2, space="PSUM"))
        ps_fft = ctx.enter_context(tc.tile_pool(name="psfft", bufs=2, space="PSUM"))

        wt_t = cpool.tile([128, SEG], F32)
        nc.sync.dma_start(wt_t[:], wt[:])
        cesc_t = cpool.tile([128, 1], F32)
        nc.sync.dma_start(cesc_t[:], cesc[:])
        cebi_t = cpool.tile([128, 1], F32)
        nc.sync.dma_start(cebi_t[:], cebi[:])
        fcat_t = cpool.tile([WIN, 34], F32)
        nc.sync.dma_start(fcat_t[:], fcat[:])
        gmat_t = cpool.tile([34, WIN], F32)
        nc.sync.dma_start(gmat_t[:], gmat[:])
        bnl_t = cpool.tile([128, 16], F32)
        nc.sync.dma_start(bnl_t[:], bnl[:])

        # ================= stage 1 =================
        x_t = []
        for k in range(2):
            xt = apool.tile([128, BPC, T0], F32, tag=f"x{k}")
            nc.sync.dma_start(xt[:], x3[:, 128 * k:128 * (k + 1), :].rearrange("b c t -> c b t"))
            x_t.append(xt)

        wm0_t = []
        for k in range(2):
            w = w1pool.tile([128, 512], F32, tag=f"wm0_{k}")
            nc.sync.dma_start(w[:], wm0[128 * k:128 * (k + 1), :])
            wm0_t.append(w)

        NCOL = BPC * T0
        h1 = []
        for m in range(4):
            pm = ps_mm.tile([128, 512], F32, tag="pconv")
            for k in range(2):
                nc.tensor.matmul(pm[:, 0:NCOL], wm0_t[k][:, 128 * m:128 * (m + 1)],
                                 x_t[k][:],
                                 start=(k == 0), stop=(k == 1))
            ht = apool.tile([128, BPC, 66], F32, tag=f"hA{m}")
            nc.vector.memset(ht[:, :, 0:1], 0.0)
            nc.vector.memset(ht[:, :, 65:66], 0.0)
            nc.scalar.activation(ht[:, :, 1:65],
                                 pm[:, 0:NCOL].rearrange("c (b t) -> c b t", b=BPC),
                                 AF.Prelu, bias=0.0, scale=1.0, alpha=0.2)
            h1.append(ht)

        hcur = h1
        for li in range(3):
            wl = []
            for k in range(4):
                w = w1pool.tile([128, 3 * 512], F32, tag=f"wmL_{k}")
                nc.sync.dma_start(w[:], wmL[li][128 * k:128 * (k + 1), :, :]
                                  .rearrange("c a o -> c (a o)"))
                wl.append(w)
            last = li == 2
            PAD = 2 if last else 1
            WID = T0 + 2 * PAD
            tagp = "hB" if li % 2 == 0 else "hA"
            hnxt = []
            for m in range(4):
                pm = ps_mm.tile([128, 512], F32, tag="pconv")
                i_mm = 0
                for k in range(4):
                    for tap in range(3):
                        nc.tensor.matmul(
                            pm[:, 0:NCOL],
                            wl[k][:, 512 * tap + 128 * m: 512 * tap + 128 * (m + 1)],
                            hcur[k][:, :, tap:tap + T0],
                            start=(i_mm == 0), stop=(i_mm == 11))
                        i_mm += 1
                ht = apool.tile([128, BPC, WID], F32,
                                tag=(f"h4_{m}" if last else f"{tagp}{m}"))
                nc.vector.memset(ht[:, :, 0:PAD], 0.0)
                nc.vector.memset(ht[:, :, PAD + T0:WID], 0.0)
                nc.scalar.activation(ht[:, :, PAD:PAD + T0],
                                     pm[:, 0:NCOL].rearrange("c (b t) -> c b t", b=BPC),
                                     AF.Prelu, bias=0.0, scale=1.0, alpha=0.2)
                hnxt.append(ht)
            hcur = hnxt
        h4 = hcur   # [128, BPC, 68], pad 2

        wfq_t = []
        for k in range(4):
            w = w1pool.tile([128, 256], F32, tag=f"wfq{k}")
            nc.sync.dma_start(w[:], wfq[128 * k:128 * (k + 1), :])
            wfq_t.append(w)
        l_sb = apool.tile([128, BPC, T0], F32, tag="l_sb")
        f_sb = apool.tile([128, BPC, T0], F32, tag="f_sb")
        for m in range(2):
            pm = ps_mm.tile([128, 512], F32, tag="pconv")
            for k in range(4):
                nc.tensor.matmul(pm[:, 0:NCOL], wfq_t[k][:, 128 * m:128 * (m + 1)],
                                 h4[k][:, :, 2:2 + T0],
                                 start=(k == 0), stop=(k == 3))
            if m == 0:
                nc.scalar.activation(l_sb[:],
                                     pm[:, 0:NCOL].rearrange("c (b t) -> c b t", b=BPC),
                                     AF.Square)
            else:
                tanh_t = apool.tile([128, BPC, T0], F32, tag="tanh")
                nc.scalar.activation(tanh_t[:],
                                     pm[:, 0:NCOL].rearrange("c (b t) -> c b t", b=BPC),
                                     AF.Tanh)
                nc.scalar.activation(f_sb[:], tanh_t[:],
                                     AF.Identity, bias=cebi_t[:], scale=cesc_t[:])

        # ================= osc prep =================
        flo_u, df_u, c_u, l2_u = [], [], [], []
        for ex in range(BPC):
            f_ex = f_sb[:, ex, :]
            l_ex = l_sb[:, ex, :]

            flo = apool.tile([128, NUNITS], F32, tag=f"flo{ex}")
            nc.vector.tensor_copy(flo[:, 0:1], f_ex[:, 0:1])
            nc.vector.tensor_copy(flo[:, 1:65], f_ex[:, 0:64])
            dfu = apool.tile([128, NUNITS], F32, tag=f"dfu{ex}")
            nc.vector.memset(dfu[:, 0:1], 0.0)
            nc.vector.memset(dfu[:, 64:65], 0.0)
            nc.vector.tensor_tensor(dfu[:, 1:64], f_ex[:, 1:64], f_ex[:, 0:63], ALU.subtract)

            l2t = apool.tile([128, NUNITS, 2], F16, tag=f"l2{ex}")
            nc.vector.tensor_copy(l2t[:, 0:1, 0], l_ex[:, 0:1])
            nc.vector.tensor_copy(l2t[:, 1:65, 0], l_ex[:, 0:64])
            nc.vector.memset(l2t[:, 0:1, 1], 0.0)
            nc.vector.memset(l2t[:, 64:65, 1], 0.0)
            nc.vector.tensor_tensor(l2t[:, 1:64, 1], l_ex[:, 1:64], l_ex[:, 0:63], ALU.subtract)

            a = f_ex[:, 0:63]
            b_ = f_ex[:, 1:64]

            def T63(tag):
                return apool.tile([128, 63], F32, tag=tag, name=tag)

            alo = T63("p_alo")
            nc.vector.tensor_tensor(alo[:], a, b_, ALU.min)
            ahi = T63("p_ahi")
            nc.vector.tensor_tensor(ahi[:], a, b_, ALU.max)
            dd = T63("p_dd")
            nc.vector.tensor_tensor(dd[:], ahi[:], alo[:], ALU.subtract)
            ddc = T63("p_ddc")
            nc.vector.tensor_scalar(ddc[:], dd[:], 1e-30, None, ALU.max)
            inv = T63("p_inv")
            nc.vector.reciprocal(inv[:], ddc[:])
            dd768 = T63("p_dd768")
            nc.vector.tensor_scalar(dd768[:], dd[:], float(1.0 / 768.0), None, ALU.mult)

            t1 = T63("p_t1")
            nc.vector.tensor_scalar(t1[:], alo[:], LO_U, -384.0, ALU.subtract, ALU.mult)
            c1 = T63("p_c1")
            nc.vector.tensor_tensor(c1[:], t1[:], inv[:], ALU.mult)
            nc.vector.tensor_scalar(c1[:], c1[:], 0.0, 384.0, ALU.max, ALU.min)
            nc.vector.tensor_scalar(c1[:], c1[:], MAGIC, MAGIC, ALU.add, ALU.subtract)
            lo_alo = T63("p_loalo")
            nc.vector.tensor_scalar(lo_alo[:], alo[:], LO_U, -1.0, ALU.subtract, ALU.mult)
            u1 = T63("p_u1")
            nc.vector.tensor_tensor(u1[:], dd768[:], c1[:], ALU.mult)
            nc.vector.tensor_tensor(u1[:], lo_alo[:], u1[:], ALU.subtract)
            s1c = T63("p_s1c")
            nc.vector.tensor_tensor(s1c[:], c1[:], u1[:], ALU.mult)

            t2 = T63("p_t2")
            nc.vector.tensor_scalar(t2[:], ahi[:], HI_U, 384.0, ALU.subtract, ALU.mult)
            c2 = T63("p_c2")
            nc.vector.tensor_tensor(c2[:], t2[:], inv[:], ALU.mult)
            nc.vector.tensor_scalar(c2[:], c2[:], 0.0, 384.0, ALU.max, ALU.min)
            nc.vector.tensor_scalar(c2[:], c2[:], MAGIC, MAGIC, ALU.add, ALU.subtract)
            ahi_hi = T63("p_ahihi")
            nc.vector.tensor_scalar(ahi_hi[:], ahi[:], HI_U, None, ALU.subtract)
            u2 = T63("p_u2")
            nc.vector.tensor_tensor(u2[:], dd768[:], c2[:], ALU.mult)
            nc.vector.tensor_tensor(u2[:], ahi_hi[:], u2[:], ALU.subtract)
            s2c = T63("p_s2c")
            nc.vector.tensor_tensor(s2c[:], c2[:], u2[:], ALU.mult)

            tall = apool.tile([128, 64], F32, tag="p_tall")
            slin = T63("p_slin")
            nc.vector.tensor_tensor(slin[:], a, b_, ALU.add)
            nc.vector.tensor_scalar(slin[:], slin[:], 192.0, None, ALU.mult)
            nc.vector.tensor_tensor(tall[:, 1:64], slin[:], s1c[:], ALU.add)
            nc.vector.tensor_tensor(tall[:, 1:64], tall[:, 1:64], s2c[:], ALU.subtract)
            nc.vector.tensor_scalar(tall[:, 0:1], f_ex[:, 0:1], LO_U, HI_U, ALU.max, ALU.min)
            nc.vector.tensor_scalar(tall[:, 0:1], tall[:, 0:1], 192.0, None, ALU.mult)
            trnd = apool.tile([128, 64], F32, tag="p_trnd")
            nc.vector.tensor_scalar(trnd[:], tall[:], MAGIC, MAGIC, ALU.add, ALU.subtract)
            nc.vector.tensor_tensor(tall[:], tall[:], trnd[:], ALU.subtract)
            cinc = apool.tile([128, 64], F32, tag="p_cinc")
            nc.vector.tensor_tensor_scan(cinc[:], tall[:], tall[:], 0.0, ALU.add, ALU.bypass)
            cu = apool.tile([128, NUNITS], F32, tag=f"cu{ex}")
            nc.vector.memset(cu[:, 0:1], 0.0)
            nc.vector.tensor_copy(cu[:, 1:65], cinc[:])

            flo_u.append(flo)
            df_u.append(dfu)
            c_u.append(cu)
            l2_u.append(l2t)

        # ================= noise branch =================
        h4_16 = []
        for k in range(4):
            h16 = apool.tile([128, BPC, 68], F16, tag=f"h416_{k}", name=f"h416_{k}")
            nc.vector.tensor_copy(h16[:], h4[k][:])
            h4_16.append(h16)
        ycur = h4_16
        TI = T0
        for li in range(4):
            TOUT = TI * 2
            WIDO = TOUT + 4
            tagp = "yA" if li % 2 == 0 else "yB"
            ynxt = [apool.tile([128, BPC, WIDO], F16, tag=f"{tagp}{m}", name=f"{tagp}{m}") for m in range(4)]
            for m in range(4):
                nc.vector.memset(ynxt[m][:, :, 0:2], 0.0)
                nc.vector.memset(ynxt[m][:, :, WIDO - 2:WIDO], 0.0)
            both = BPC * TI <= 512
            for eo in range(2):
                wsrc = wn0 if li == 0 else wnl[li - 1]
                wgt = w2pool.tile([128, 4, 4 * 512], F16, tag="wn16")
                for k in range(4):
                    nc.sync.dma_start(wgt[:, k, :],
                                      wsrc[eo, 128 * k:128 * (k + 1), :, :]
                                      .rearrange("c a o -> c (a o)"))
                wg = [wgt[:, k, :] for k in range(4)]
                for m in range(4):
                    bias_ap = bnl_t[:, 4 * li + m:4 * li + m + 1]
                    ex_sets = [None] if both else list(range(BPC))
                    for ex in ex_sets:
                        pm = ps_mm.tile([128, 512], F32, tag="pconv")
                        ncols = BPC * TI if both else TI
                        i_mm = 0
                        for k in range(4):
                            wk = wg[k]
                            for tap in range(4):
                                off = tap + eo
                                if both:
                                    rhs = ycur[k][:, :, off:off + TI]
                                else:
                                    rhs = ycur[k][:, ex, off:off + TI]
                                lhs = wk[:, 512 * tap + 128 * m:512 * tap + 128 * (m + 1)]
                                nc.tensor.matmul(pm[:, 0:ncols], lhs, rhs,
                                                 start=(i_mm == 0), stop=(i_mm == 15))
                                i_mm += 1
                        if both:
                            dst = ynxt[m][:, :, 2 + eo:2 + eo + 2 * TI:2]
                            srcp = pm[:, 0:ncols].rearrange("c (b t) -> c b t", b=BPC)
                        else:
                            dst = ynxt[m][:, ex, 2 + eo:2 + eo + 2 * TI:2]
                            srcp = pm[:, 0:ncols]
                        nc.scalar.activation(dst, srcp, AF.Prelu,
                                             bias=bias_ap, scale=1.0, alpha=0.2)
            ycur = ynxt
            TI = TOUT

        wh_t = w2pool.tile([128, 4, 34], F16, tag="wnh")
        for k in range(4):
            nc.sync.dma_start(wh_t[:, k, :], wnh[128 * k:128 * (k + 1), :])
        nl_sb = []
        for ex in range(BPC):
            nlt = apool.tile([34, FRAMES], F32, tag=f"nl{ex}")
            for half in range(2):
                pm = ps_fft.tile([34, 512], F32, tag="pfft")
                for k in range(4):
                    nc.tensor.matmul(pm[:],
                                     wh_t[:, k, :],
                                     ycur[k][:, ex, 2 + 512 * half:2 + 512 * (half + 1)],
                                     start=(k == 0), stop=(k == 3))
                nc.scalar.activation(nlt[:, 512 * half:512 * (half + 1)], pm[:], AF.Square)
            nl_sb.append(nlt)

        # ================= noise FFT =================
        for ex in range(BPC):
            nzt = fpool.tile([WIN, FRAMES], F32, tag="nz")
            nc.sync.dma_start_transpose(nzt[:], noi[ex, :, :])
            nz2 = fpool.tile([WIN, FRAMES], F32, tag="nz2")
            nc.scalar.activation(nz2[:], nzt[:], AF.Copy, bias=-1.0, scale=2.0)
            fcs = fpool.tile([34, FRAMES], F32, tag="fcs")
            for half in range(2):
                pm = ps_fft.tile([34, 512], F32, tag="pfft")
                nc.tensor.matmul(pm[:], fcat_t[:], nz2[:, 512 * half:512 * (half + 1)],
                                 start=True, stop=True)
                nc.vector.tensor_tensor(fcs[:, 512 * half:512 * (half + 1)], pm[:],
                                        nl_sb[ex][:, 512 * half:512 * (half + 1)], ALU.mult)
            frsA = fpool.tile([16, FRAMES], F32, tag="frsA")
            frsB = fpool.tile([16, FRAMES], F32, tag="frsB")
            for half in range(2):
                pm = ps_fft.tile([34, 512], F32, tag="pfft")
                nc.tensor.matmul(pm[0:16, :], gmat_t[:, 0:16],
                                 fcs[:, 512 * half:512 * (half + 1)],
                                 start=True, stop=True)
                nc.scalar.copy(frsA[:, 512 * half:512 * (half + 1)], pm[0:16, :])
                pm2 = ps_fft.tile([34, 512], F32, tag="pfft")
                nc.tensor.matmul(pm2[0:16, :], gmat_t[:, 16:32],
                                 fcs[:, 512 * half:512 * (half + 1)],
                                 start=True, stop=True)
                nc.scalar.copy(frsB[:, 512 * half:512 * (half + 1)], pm2[0:16, :])
            nsb = fpool.tile([16, FRAMES], F32, tag="nsb")
            nc.vector.tensor_copy(nsb[:, 0:1], frsA[:, 0:1])
            nc.vector.tensor_tensor(nsb[:, 1:FRAMES], frsA[:, 1:FRAMES],
                                    frsB[:, 0:FRAMES - 1], ALU.add)
            nc.sync.dma_start(n_out[16 * ex:16 * (ex + 1), :], nsb[:])

        # ================= osc bank =================
        two_pi = float(2.0 * np.pi)
        units = [(0, 0, EDGE)]
        for s in range(NSEG):
            units.append((1 + s, EDGE + SEG * s, SEG))
        units.append((NUNITS - 1, UP_LEN - EDGE, EDGE))
        for ex in range(BPC):
            for g0 in range(0, NUNITS, 4):
                group = units[g0:g0 + 4]
                pm4 = ps_osc.tile([128, SEG], F32, tag="pm4")
                for gi, (j, c0, wdt) in enumerate(group):
                    fu = opool.tile([128, SEG], F32, tag="fu")
                    nc.scalar.activation(fu[:, 0:wdt], wt_t[:, 0:wdt], AF.Identity,
                                         bias=flo_u[ex][:, j:j + 1],
                                         scale=df_u[ex][:, j:j + 1])
                    ph = opool.tile([128, SEG], F32, tag="ph")
                    nc.vector._custom_dve(
                        osc_op, out=ph[:, 0:wdt], in0=fu[:, 0:wdt],
                        in1=c_u[ex][:, j:j + 1].to_broadcast((128, wdt)),
                        s0=LO_U, s1=HI_U, imm2=MAGIC)
                    s16 = opool.tile([128, SEG], F16, tag="s16")
                    nc.scalar.activation(s16[:, 0:wdt], ph[:, 0:wdt], AF.Sin,
                                         bias=0.0, scale=two_pi)
                    nc.tensor.matmul(pm4[32 * gi:32 * gi + 2, 0:wdt],
                                     l2_u[ex][:, j, :], s16[:, 0:wdt],
                                     start=True, stop=True,
                                     tile_position=(0, 32 * gi))
                hm = hpool.tile([128, SEG], F32, tag="hm")
                nc.vector.tensor_copy(hm[:], pm4[:])
                for gi, (j, c0, wdt) in enumerate(group):
                    nc.sync.dma_start(h_out[2 * ex:2 * ex + 2, c0:c0 + wdt],
                                      hm[32 * gi:32 * gi + 2, 0:wdt])

    nc.compile()
    _BUILD_CACHE["nc"] = nc
    return nc


_W_PAT = None


def _wpat():
    global _W_PAT
    if _W_PAT is None:
        w = np.zeros(UP_LEN, np.float32)
        kk = ((np.arange(SEG) + 0.5) / SEG).astype(np.float32)
        for s in range(NSEG):
            w[EDGE + SEG * s: EDGE + SEG * (s + 1)] = kk
        _W_PAT = w
    return _W_PAT


def _prep_shared(inputs):
    d = {}
    d["wm0"] = np.ascontiguousarray(inputs["w_main0"][:, :, 0].T)
    for i in (1, 2, 3):
        d[f"wm{i}"] = np.ascontiguousarray(np.asarray(inputs[f"w_main{i}"]).transpose(1, 2, 0))
    d["wfq"] = np.ascontiguousarray(inputs["w_freq"][:, :, 0].T)
    for l in range(4):
        W = np.asarray(inputs[f"w_nl{l}"])
        We = np.stack([W[:, :, 0], W[:, :, 1] + W[:, :, 2],
                       W[:, :, 3] + W[:, :, 4], W[:, :, 5] + W[:, :, 6]], -1)
        Wo = np.stack([W[:, :, 0] + W[:, :, 1], W[:, :, 2] + W[:, :, 3],
                       W[:, :, 4] + W[:, :, 5], W[:, :, 6]], -1)
        arr = np.stack([We.transpose(1, 2, 0), Wo.transpose(1, 2, 0)], 0)
        key = "wn0" if l == 0 else f"wnl{l}"
        d[key] = np.ascontiguousarray(arr.astype(np.float16))
    wh = np.asarray(inputs["w_noise_loud"])[:, :, 0].T          # [512, 17]
    d["wnh"] = np.ascontiguousarray(
        np.concatenate([wh, wh], 1).astype(np.float16))
    bn = np.zeros((128, 16), np.float32)
    for l in range(4):
        bl = np.asarray(inputs[f"b_nl{l}"]).reshape(4, 128)
        for m in range(4):
            bn[:, 4 * l + m] = bl[m]
    d["bnl"] = bn
    d["wt"] = np.ascontiguousarray(
        np.broadcast_to(((np.arange(SEG) + 0.5) / SEG).astype(np.float32), (128, SEG)))
    k = np.arange(WIN)[:, None].astype(np.float64)
    j = np.arange(17)[None, :].astype(np.float64)
    fre = np.cos(-2 * np.pi * k * j / WIN) / np.sqrt(WIN)
    fim = np.sin(-2 * np.pi * k * j / WIN) / np.sqrt(WIN)
    d["fcat"] = np.ascontiguousarray(np.concatenate([fre, fim], 1).astype(np.float32))
    t = np.arange(WIN)[None, :].astype(np.float64)
    jj = np.arange(17)[:, None].astype(np.float64)
    wgt = np.where((jj == 0) | (jj == 16), 1.0, 2.0)
    gre = wgt * np.cos(2 * np.pi * jj * t / WIN) / np.sqrt(WIN)
    gim = -wgt * np.sin(2 * np.pi * jj * t / WIN) / np.sqrt(WIN)
    d["gmat"] = np.ascontiguousarray(np.concatenate([gre, gim], 0).astype(np.float32))
    d["cesc"] = (0.5 * _ERBS / SR).astype(np.float32).reshape(128, 1)
    d["cebi"] = (_CENTERS / SR).astype(np.float32).reshape(128, 1)
    return d


def _in_maps(inputs):
    shared = _prep_shared(inputs)
    x = np.asarray(inputs["x"], np.float32)
    noise = np.asarray(inputs["noise"], np.float32)
    maps = []
    for c in range(NCORES):
        m = dict(shared)
        m["x3"] = np.ascontiguousarray(x[BPC * c:BPC * (c + 1)])
        m["noi"] = np.ascontiguousarray(noise[BPC * c:BPC * (c + 1)])
        maps.append(m)
    return maps


def _assemble(results):
    wpat = _wpat()
    out = np.empty((B, 1, TOTAL), np.float32)
    for c in range(NCORES):
        h_o = results[c]["h_out"]
        n_o = results[c]["n_out"]
        for ex in range(BPC):
            bidx = BPC * c + ex
            sig = h_o[2 * ex] + wpat * h_o[2 * ex + 1]
            nzf = np.ascontiguousarray(n_o[16 * ex:16 * (ex + 1)].T).reshape(TOTAL)
            sig[CROP:CROP + TOTAL] += nzf
            mx = np.abs(sig).max()
            out[bidx, 0] = sig[CROP:CROP + TOTAL] / (mx + np.float32(1e-8))
    return out


def kernel(**inputs) -> np.ndarray:
    nc = _build_program()
    maps = _in_maps(inputs)
    res = bass_utils.run_bass_kernel_spmd(nc, maps, core_ids=list(range(NCORES)))
    return _assemble([res.results[c] for c in range(NCORES)])
